# revision 7
# baseline (speedup 1.0000x reference)
"""Depth rasterization (MANO hand z-buffer @ 640x640 -> bilinear 128x128).

Key identities exploited:
  * jax.image.resize(640->128, linear, antialias=False) samples input coords
    5*j + 2.0 exactly -> output[i, j] == raster[5i+2, 5j+2]. Only the 128x128
    decimated pixel grid (centers x = 5j+2.5, y = 5i+2.5) is rasterized: a
    25x reduction vs the reference's 640x640 raster.
  * Edge functions and barycentric depth are affine in pixel coords, so each
    triangle yields four planes over the basis (j, i, 1):
      P_k = OFF - S * sign(area) * e_k     (k = 0,1,2 penalty planes)
      W   = (e0*z0 + e1*z1 + e2*z2) / area (depth plane)
    key(p, f) = max(P0, P1, P2, W) equals the interpolated depth when p is
    inside triangle f and is >= OFF (>> the 100 clamp) outside; the z-buffer
    is zbuf(p) = min(100, min_f key(p, f)).
  * Plane evaluation is a K=9 bf16 matmul (coefficients split into 3 bf16
    limbs; the (j, i, 1) basis is exact in bf16, giving fp32-grade accuracy
    at full bf16 PE speed); the 4 planes run concurrently in the PE via
    32-row tile_position groups.
  * Per 16x8-pixel tile, candidates are bbox-filtered and hierarchical-z
    pruned on the host (exact: a candidate whose minimum possible depth over
    the tile exceeds the best fully-covering candidate's maximum depth can
    never win). Tiles are assigned to kernel "slots" sorted by candidate
    count; slot capacities (compile-time) are the per-rank maxima across all
    8 cores, so every tile fits exactly - no truncation for any input.

Sharding: 8 cores = 4 batch elements x 2 half-images (64 tiles of 16x8 px).
"""

import numpy as np
import ml_dtypes

import concourse.bacc as bacc
import concourse.mybir as mybir
import concourse.tile as tile
from concourse.bass_utils import run_bass_kernel_spmd

_B, _V, _F = 4, 778, 1538
_H = _W = 128
_NT = 64           # tiles (slots) per core
_TJ, _TI = 16, 8   # tile size in output pixels (x, y)
_OFF = 1000.0      # penalty-plane offset (>> 100 clamp)
_S = 1.0e9         # penalty scale
_BIGC = 1.0e7      # plane constant for padding/invalid
_CLAMP = 100.0
_COVER_MARGIN = 1.0    # e*s margin (e-units) for the full-cover test
_BOUND_MARGIN = 1e-3   # depth margin for the prune bound

_F32 = mybir.dt.float32
_BF16 = mybir.dt.bfloat16
_BF16_NP = ml_dtypes.bfloat16

_NC_CACHE = {}
PROFILE = {}


def _build_nc(caps):
    """caps: tuple of 64 slot capacities (each <= 512, multiple of 32)."""
    total = int(sum(caps))
    nc = bacc.Bacc("TRN2", target_bir_lowering=False, debug=False, num_devices=8)
    coef_d = nc.dram_tensor("coef", [36, total], _BF16, kind="ExternalInput")
    pix_d = nc.dram_tensor("pix", [36, _NT * 128], _BF16, kind="ExternalInput")
    out_d = nc.dram_tensor("out", [128, _NT], _F32, kind="ExternalOutput")

    with tile.TileContext(nc) as tc:
        with (
            tc.tile_pool(name="const", bufs=1) as cpool,
            tc.tile_pool(name="scr", bufs=2) as spool,
            tc.tile_pool(name="ps", bufs=8, space="PSUM") as ppool,
        ):
            pixt = cpool.tile([128, _NT * 128], _BF16)
            coeft = cpool.tile([128, total], _BF16)
            zmin = cpool.tile([128, _NT], _F32)
            for k in range(4):
                nc.sync.dma_start(pixt[32 * k : 32 * k + 9, :], pix_d.ap()[9 * k : 9 * k + 9, :])
                nc.sync.dma_start(coeft[32 * k : 32 * k + 9, :], coef_d.ap()[9 * k : 9 * k + 9, :])

            off = 0
            for s in range(_NT):
                cap = int(caps[s])
                cs = slice(off, off + cap)
                off += cap
                cols = slice(s * 128, (s + 1) * 128)
                pa0 = ppool.tile([128, 512], _F32, tag="ps", name="pa0")
                pa1 = ppool.tile([128, 512], _F32, tag="ps", name="pa1")
                pb2 = ppool.tile([128, 512], _F32, tag="ps", name="pb2")
                pw = ppool.tile([128, 512], _F32, tag="ps", name="pw")
                nc.tensor.matmul(pa0[:, :cap], pixt[0:9, cols], coeft[0:9, cs],
                                 start=True, stop=True, tile_position=(0, 0))
                nc.tensor.matmul(pa1[:, :cap], pixt[32:41, cols], coeft[32:41, cs],
                                 start=True, stop=True, tile_position=(32, 0))
                nc.tensor.matmul(pb2[:, :cap], pixt[64:73, cols], coeft[64:73, cs],
                                 start=True, stop=True, tile_position=(64, 0))
                nc.tensor.matmul(pw[:, :cap], pixt[96:105, cols], coeft[96:105, cs],
                                 start=True, stop=True, tile_position=(96, 0))
                # DVE reads at most one PSUM operand per op: Scalar engine
                # pulls P0 to SBUF in parallel.
                t0 = spool.tile([128, 512], _F32, tag="t0", name="t0")
                nc.scalar.copy(t0[:, :cap], pa0[:, :cap])
                t01 = spool.tile([128, 512], _F32, tag="t01", name="t01")
                nc.vector.tensor_tensor(t01[:, :cap], t0[:, :cap], pa1[:, :cap],
                                        op=mybir.AluOpType.max)
                t012 = spool.tile([128, 512], _F32, tag="t012", name="t012")
                nc.vector.tensor_tensor(t012[:, :cap], t01[:, :cap], pb2[:, :cap],
                                        op=mybir.AluOpType.max)
                keyt = spool.tile([128, 512], _F32, tag="key", name="keyt")
                nc.vector.tensor_tensor(keyt[:, :cap], t012[:, :cap], pw[:, :cap],
                                        op=mybir.AluOpType.max)
                nc.vector.tensor_reduce(zmin[:, s : s + 1], keyt[:, :cap],
                                        axis=mybir.AxisListType.X, op=mybir.AluOpType.min)

            zclamp = cpool.tile([128, _NT], _F32)
            nc.vector.tensor_scalar_min(zclamp[:], zmin[:], _CLAMP)
            nc.sync.dma_start(out_d.ap(), zclamp[:])

    nc.compile()
    return nc


def _get_nc(caps):
    if caps not in _NC_CACHE:
        _NC_CACHE[caps] = _build_nc(caps)
    return _NC_CACHE[caps]


def _planes64(vertices, faces):
    """Full-precision planes on basis (j, i, 1): [B, 4, 3, F] f64 + aux."""
    v64 = vertices.astype(np.float64)
    fidx = np.asarray(faces).astype(np.int64).reshape(-1)
    fv = v64[:, fidx, :].reshape(_B, _F, 3, 3)
    x0, y0, z0 = fv[:, :, 0, 0], fv[:, :, 0, 1], fv[:, :, 0, 2]
    x1, y1, z1 = fv[:, :, 1, 0], fv[:, :, 1, 1], fv[:, :, 1, 2]
    x2, y2, z2 = fv[:, :, 2, 0], fv[:, :, 2, 1], fv[:, :, 2, 2]

    # area exactly as the reference computes it (float32 ops)
    v32 = vertices.astype(np.float32)
    fv32 = v32[:, fidx, :].reshape(_B, _F, 3, 3)
    xa, ya = fv32[:, :, 0, 0], fv32[:, :, 0, 1]
    xb, yb = fv32[:, :, 1, 0], fv32[:, :, 1, 1]
    xc, yc = fv32[:, :, 2, 0], fv32[:, :, 2, 1]
    area32 = (xb - xa) * (yc - ya) - (yb - ya) * (xc - xa)
    s = np.sign(area32).astype(np.float64)
    valid = np.abs(area32) > 1e-12

    A0 = -(y2 - y1); B0 = x2 - x1; C0 = (y2 - y1) * x1 - (x2 - x1) * y1
    A1 = -(y0 - y2); B1 = x0 - x2; C1 = (y0 - y2) * x2 - (x0 - x2) * y2
    A2 = -(y1 - y0); B2 = x1 - x0; C2 = (y1 - y0) * x0 - (x1 - x0) * y0

    area64 = np.where(valid, area32.astype(np.float64), 1.0)
    Aw = (z0 * A0 + z1 * A1 + z2 * A2) / area64
    Bw = (z0 * B0 + z1 * B1 + z2 * B2) / area64
    Cw = (z0 * C0 + z1 * C1 + z2 * C2) / area64

    planes = np.zeros((_B, 4, 3, _F), np.float64)
    raw = [
        (-_S * s * A0, -_S * s * B0, _OFF - _S * s * C0),
        (-_S * s * A1, -_S * s * B1, _OFF - _S * s * C1),
        (-_S * s * A2, -_S * s * B2, _OFF - _S * s * C2),
        (Aw, Bw, Cw),
    ]
    for k, (a, b, c) in enumerate(raw):
        a = np.where(valid, a, 0.0)
        b = np.where(valid, b, 0.0)
        c = np.where(valid, c, _BIGC)
        # basis change px = 5j + 2.5, py = 5i + 2.5 -> (j, i, 1)
        planes[:, k, 0] = 5.0 * a
        planes[:, k, 1] = 5.0 * b
        planes[:, k, 2] = 2.5 * a + 2.5 * b + c

    xsmin = fv[..., 0].min(2); xsmax = fv[..., 0].max(2)
    ysmin = fv[..., 1].min(2); ysmax = fv[..., 1].max(2)
    zmin_tri = fv[..., 2].min(2)
    return planes, valid, xsmin, xsmax, ysmin, ysmax, zmin_tri


def _split3(c64):
    """[rows, n] f64 -> [3*rows, n] bf16 (hi/mid/lo limbs)."""
    hi = c64.astype(_BF16_NP).astype(np.float64)
    mid = (c64 - hi).astype(_BF16_NP).astype(np.float64)
    lo = (c64 - hi - mid).astype(_BF16_NP)
    return hi.astype(_BF16_NP), mid.astype(_BF16_NP), lo


def _prepare(vertices, faces):
    """Host binning/pruning/packing. Returns (caps, in_maps data, slot maps)."""
    planes, valid, xsmin, xsmax, ysmin, ysmax, zmin_tri = _planes64(vertices, faces)
    ntj = _W // _TJ

    kept_lists = []  # [core][slot_ordering later] per-tile candidate arrays
    for c in range(8):
        b, h = c // 2, c % 2
        P = planes[b]  # [4, 3, F]
        tiles = []
        for t in range(_NT):
            tj, ti = t % ntj, t // ntj
            j0, i0 = tj * _TJ, ti * _TI + 64 * h
            xlo, xhi = 5 * j0 + 2.5, 5 * (j0 + _TJ - 1) + 2.5
            ylo, yhi = 5 * i0 + 2.5, 5 * (i0 + _TI - 1) + 2.5
            cand = np.where(valid[b] & (xsmax[b] >= xlo) & (xsmin[b] <= xhi)
                            & (ysmax[b] >= ylo) & (ysmin[b] <= yhi))[0]
            if len(cand) == 0:
                tiles.append((t, np.empty(0, np.int64)))
                continue
            corners = np.array(
                [[j0, i0, 1], [j0 + _TJ - 1, i0, 1],
                 [j0, i0 + _TI - 1, 1], [j0 + _TJ - 1, i0 + _TI - 1, 1]],
                np.float64)
            Wc = corners @ P[3][:, cand]           # [4, nc]
            zlo = np.maximum(Wc.min(0), zmin_tri[b][cand])
            covers = np.ones(len(cand), bool)
            for k in range(3):
                Pc = corners @ P[k][:, cand]
                covers &= (Pc <= _OFF - _S * _COVER_MARGIN).all(axis=0)
            bound = (Wc.max(0)[covers].min() + _BOUND_MARGIN) if covers.any() else np.inf
            keep = zlo <= bound
            order = cand[keep][np.argsort(zlo[keep])]
            tiles.append((t, order))
        kept_lists.append(tiles)

    # sort each core's tiles by kept desc -> slots; per-rank max across cores
    slot_orders = []
    for c in range(8):
        order = sorted(range(_NT), key=lambda t: -len(kept_lists[c][t][1]))
        slot_orders.append(order)
    caps = []
    for s in range(_NT):
        m = max(len(kept_lists[c][slot_orders[c][s]][1]) for c in range(8))
        caps.append(max(32, ((m + 31) // 32) * 32))
    caps = tuple(int(min(c, 512)) for c in caps)
    assert all(caps[i] >= caps[i + 1] for i in range(_NT - 1))
    total = sum(caps)

    in_maps = []
    for c in range(8):
        b, h = c // 2, c % 2
        coef_g64 = np.zeros((12, total), np.float64)
        coef_g64[np.arange(4) * 3 + 2, :] = _BIGC  # dummy padding planes
        pix_g = np.zeros((36, _NT * 128), np.float32)
        off = 0
        for s in range(_NT):
            t = slot_orders[c][s]
            idx = kept_lists[c][t][1]
            n = len(idx)
            for k in range(4):
                coef_g64[3 * k : 3 * k + 3, off : off + n] = planes[b, k][:, idx]
            off += caps[s]
            tj, ti = t % (_W // _TJ), t // (_W // _TJ)
            j0, i0 = tj * _TJ, ti * _TI + 64 * h
            jj = j0 + np.tile(np.arange(_TJ, dtype=np.float32), _TI)
            ii = i0 + np.repeat(np.arange(_TI, dtype=np.float32), _TJ)
            for r in range(12):
                pix_g[3 * r + 0, s * 128 : (s + 1) * 128] = jj
                pix_g[3 * r + 1, s * 128 : (s + 1) * 128] = ii
                pix_g[3 * r + 2, s * 128 : (s + 1) * 128] = 1.0
        coef36 = np.zeros((36, total), _BF16_NP)
        for k in range(4):
            hi, mid, lo = _split3(coef_g64[3 * k : 3 * k + 3])
            coef36[9 * k + 0 : 9 * k + 3] = hi
            coef36[9 * k + 3 : 9 * k + 6] = mid
            coef36[9 * k + 6 : 9 * k + 9] = lo
        in_maps.append({"coef": coef36, "pix": pix_g.astype(_BF16_NP)})
    return caps, in_maps, slot_orders


def kernel(vertices, faces):
    vertices = np.asarray(vertices)
    faces = np.asarray(faces)
    caps, in_maps, slot_orders = _prepare(vertices, faces)

    nc = _get_nc(caps)
    kw = dict(PROFILE.get("run_kwargs", {}))
    res = run_bass_kernel_spmd(nc, in_maps, list(range(8)), **kw)
    PROFILE["last_result"] = res

    ntj = _W // _TJ
    out = np.empty((_B, _H, _W), np.float32)
    for c in range(8):
        b, h = c // 2, c % 2
        z = res.results[c]["out"]  # [128, NT]
        for s in range(_NT):
            t = slot_orders[c][s]
            tj, ti = t % ntj, t // ntj
            j0, i0 = tj * _TJ, ti * _TI + 64 * h
            out[b, i0 : i0 + _TI, j0 : j0 + _TJ] = z[:, s].reshape(_TI, _TJ)
    return out


# revision 10
# speedup vs baseline: 1.1962x; 1.1962x over previous
"""Depth rasterization (MANO hand z-buffer @ 640x640 -> bilinear 128x128).

Key identities exploited:
  * jax.image.resize(640->128, linear, antialias=False) samples input coords
    5*j + 2.0 exactly -> output[i, j] == raster[5i+2, 5j+2]. Only the 128x128
    decimated pixel grid (centers x = 5j+2.5, y = 5i+2.5) is rasterized: a
    25x reduction vs the reference's 640x640 raster.
  * Edge functions and barycentric depth are affine in pixel coords, so each
    triangle yields four planes over the basis (j, i, 1):
      P_k = OFF - S * sign(area) * e_k     (k = 0,1,2 penalty planes)
      W   = (e0*z0 + e1*z1 + e2*z2) / area (depth plane)
    key(p, f) = max(P0, P1, P2, W) equals the interpolated depth when p is
    inside triangle f and is >= OFF (>> the 100 clamp) outside; the z-buffer
    is zbuf(p) = min(100, min_f key(p, f)).
  * Plane evaluation is a K=9 bf16 matmul (coefficients split into 3 bf16
    limbs; the (j, i, 1) basis is exact in bf16, giving fp32-grade accuracy
    at full bf16 PE speed); the 4 planes run concurrently in the PE via
    32-row tile_position groups.
  * Per 16x8-pixel tile, candidates are bbox-filtered and hierarchical-z
    pruned on the host (exact: a candidate whose minimum possible depth over
    the tile exceeds the best fully-covering candidate's maximum depth can
    never win). Tiles are assigned to kernel "slots" sorted by candidate
    count; slot capacities (compile-time) are the per-rank maxima across all
    8 cores, so every tile fits exactly - no truncation for any input.

Sharding: 8 cores = 4 batch elements x 2 half-images (64 tiles of 16x8 px).
"""

import numpy as np
import ml_dtypes

import concourse.bacc as bacc
import concourse.mybir as mybir
import concourse.tile as tile
from concourse.bass_utils import run_bass_kernel_spmd

_B, _V, _F = 4, 778, 1538
_H = _W = 128
_NT = 64           # tiles (slots) per core
_TJ, _TI = 16, 8   # tile size in output pixels (x, y)
_OFF = 1000.0      # penalty-plane offset (>> 100 clamp)
_S = 1.0e9         # penalty scale
_BIGC = 1.0e7      # plane constant for padding/invalid
_CLAMP = 100.0
_COVER_MARGIN = 1.0    # e*s margin (e-units) for the full-cover test
_BOUND_MARGIN = 1e-3   # depth margin for the prune bound

_F32 = mybir.dt.float32
_BF16 = mybir.dt.bfloat16
_BF16_NP = ml_dtypes.bfloat16

_NC_CACHE = {}
PROFILE = {}


def _build_nc(caps):
    """caps: tuple of 64 slot capacities (each <= 512, multiple of 32)."""
    total = int(sum(caps))
    ngroups = 8
    gsl = [slice(g * 8, (g + 1) * 8) for g in range(ngroups)]
    gcap = [int(sum(caps[g * 8 : (g + 1) * 8])) for g in range(ngroups)]
    goff = [int(sum(gcap[:g])) for g in range(ngroups)]
    nc = bacc.Bacc("TRN2", target_bir_lowering=False, debug=False, num_devices=8)
    # single dense [128, ...] input: cols [0:total] coef, then NT*128 pix;
    # rows live at partitions {32k .. 32k+8} (4 row-groups x 9 limbs).
    data_d = nc.dram_tensor("data", [128, total + _NT * 128], _BF16, kind="ExternalInput")
    out_d = nc.dram_tensor("out", [128, _NT], _F32, kind="ExternalOutput")

    with tile.TileContext(nc) as tc:
        with (
            tc.tile_pool(name="const", bufs=1) as cpool,
            tc.tile_pool(name="scr", bufs=3) as spool,
            tc.tile_pool(name="ps", bufs=8, space="PSUM") as ppool,
        ):
            zmin = cpool.tile([128, _NT], _F32)
            # per-group tiles so DMA of group g+1 overlaps compute on group g
            ctiles, ptiles = [], []
            for g in range(ngroups):
                ct = cpool.tile([128, gcap[g]], _BF16, name=f"coef{g}")
                pt = cpool.tile([128, 8 * 128], _BF16, name=f"pix{g}")
                nc.sync.dma_start(ct[:], data_d.ap()[:, goff[g] : goff[g] + gcap[g]])
                nc.sync.dma_start(pt[:], data_d.ap()[:, total + g * 1024 : total + (g + 1) * 1024])
                ctiles.append(ct)
                ptiles.append(pt)

            for s in range(_NT):
                g, si = s // 8, s % 8
                cap = int(caps[s])
                o = int(sum(caps[g * 8 : s]))
                cs = slice(o, o + cap)
                cols = slice(si * 128, (si + 1) * 128)
                coeft, pixt = ctiles[g], ptiles[g]
                pa0 = ppool.tile([128, 512], _F32, tag="ps", name="pa0")
                pa1 = ppool.tile([128, 512], _F32, tag="ps", name="pa1")
                pb2 = ppool.tile([128, 512], _F32, tag="ps", name="pb2")
                pw = ppool.tile([128, 512], _F32, tag="ps", name="pw")
                nc.tensor.matmul(pa0[:, :cap], pixt[0:9, cols], coeft[0:9, cs],
                                 start=True, stop=True, tile_position=(0, 0))
                nc.tensor.matmul(pa1[:, :cap], pixt[32:41, cols], coeft[32:41, cs],
                                 start=True, stop=True, tile_position=(32, 0))
                nc.tensor.matmul(pb2[:, :cap], pixt[64:73, cols], coeft[64:73, cs],
                                 start=True, stop=True, tile_position=(64, 0))
                nc.tensor.matmul(pw[:, :cap], pixt[96:105, cols], coeft[96:105, cs],
                                 start=True, stop=True, tile_position=(96, 0))
                # DVE reads at most one PSUM operand per op: ScalarE pulls
                # P0 and W to SBUF; GpSimd does the SBUF-only final max.
                t0 = spool.tile([128, 512], _F32, tag="t0", name="t0")
                nc.scalar.copy(t0[:, :cap], pa0[:, :cap])
                tw = spool.tile([128, 512], _F32, tag="tw", name="tw")
                nc.scalar.copy(tw[:, :cap], pw[:, :cap])
                t01 = spool.tile([128, 512], _F32, tag="t01", name="t01")
                nc.vector.tensor_tensor(t01[:, :cap], t0[:, :cap], pa1[:, :cap],
                                        op=mybir.AluOpType.max)
                t2w = spool.tile([128, 512], _F32, tag="t2w", name="t2w")
                nc.vector.tensor_tensor(t2w[:, :cap], tw[:, :cap], pb2[:, :cap],
                                        op=mybir.AluOpType.max)
                keyt = spool.tile([128, 512], _F32, tag="key", name="keyt")
                nc.vector.tensor_tensor(keyt[:, :cap], t01[:, :cap], t2w[:, :cap],
                                        op=mybir.AluOpType.max)
                nc.vector.tensor_reduce(zmin[:, s : s + 1], keyt[:, :cap],
                                        axis=mybir.AxisListType.X, op=mybir.AluOpType.min)

            zclamp = cpool.tile([128, _NT], _F32)
            nc.vector.tensor_scalar_min(zclamp[:], zmin[:], _CLAMP)
            nc.sync.dma_start(out_d.ap(), zclamp[:])

    nc.compile()
    return nc


def _get_nc(caps):
    if caps not in _NC_CACHE:
        _NC_CACHE[caps] = _build_nc(caps)
    return _NC_CACHE[caps]


def _planes64(vertices, faces):
    """Full-precision planes on basis (j, i, 1): [B, 4, 3, F] f64 + aux."""
    v64 = vertices.astype(np.float64)
    fidx = np.asarray(faces).astype(np.int64).reshape(-1)
    fv = v64[:, fidx, :].reshape(_B, _F, 3, 3)
    x0, y0, z0 = fv[:, :, 0, 0], fv[:, :, 0, 1], fv[:, :, 0, 2]
    x1, y1, z1 = fv[:, :, 1, 0], fv[:, :, 1, 1], fv[:, :, 1, 2]
    x2, y2, z2 = fv[:, :, 2, 0], fv[:, :, 2, 1], fv[:, :, 2, 2]

    # area exactly as the reference computes it (float32 ops)
    v32 = vertices.astype(np.float32)
    fv32 = v32[:, fidx, :].reshape(_B, _F, 3, 3)
    xa, ya = fv32[:, :, 0, 0], fv32[:, :, 0, 1]
    xb, yb = fv32[:, :, 1, 0], fv32[:, :, 1, 1]
    xc, yc = fv32[:, :, 2, 0], fv32[:, :, 2, 1]
    area32 = (xb - xa) * (yc - ya) - (yb - ya) * (xc - xa)
    s = np.sign(area32).astype(np.float64)
    valid = np.abs(area32) > 1e-12

    A0 = -(y2 - y1); B0 = x2 - x1; C0 = (y2 - y1) * x1 - (x2 - x1) * y1
    A1 = -(y0 - y2); B1 = x0 - x2; C1 = (y0 - y2) * x2 - (x0 - x2) * y2
    A2 = -(y1 - y0); B2 = x1 - x0; C2 = (y1 - y0) * x0 - (x1 - x0) * y0

    area64 = np.where(valid, area32.astype(np.float64), 1.0)
    Aw = (z0 * A0 + z1 * A1 + z2 * A2) / area64
    Bw = (z0 * B0 + z1 * B1 + z2 * B2) / area64
    Cw = (z0 * C0 + z1 * C1 + z2 * C2) / area64

    planes = np.zeros((_B, 4, 3, _F), np.float64)
    raw = [
        (-_S * s * A0, -_S * s * B0, _OFF - _S * s * C0),
        (-_S * s * A1, -_S * s * B1, _OFF - _S * s * C1),
        (-_S * s * A2, -_S * s * B2, _OFF - _S * s * C2),
        (Aw, Bw, Cw),
    ]
    for k, (a, b, c) in enumerate(raw):
        a = np.where(valid, a, 0.0)
        b = np.where(valid, b, 0.0)
        c = np.where(valid, c, _BIGC)
        # basis change px = 5j + 2.5, py = 5i + 2.5 -> (j, i, 1)
        planes[:, k, 0] = 5.0 * a
        planes[:, k, 1] = 5.0 * b
        planes[:, k, 2] = 2.5 * a + 2.5 * b + c

    xsmin = fv[..., 0].min(2); xsmax = fv[..., 0].max(2)
    ysmin = fv[..., 1].min(2); ysmax = fv[..., 1].max(2)
    zmin_tri = fv[..., 2].min(2)
    return planes, valid, xsmin, xsmax, ysmin, ysmax, zmin_tri


def _split3(c64):
    """[rows, n] f64 -> [3*rows, n] bf16 (hi/mid/lo limbs)."""
    hi = c64.astype(_BF16_NP).astype(np.float64)
    mid = (c64 - hi).astype(_BF16_NP).astype(np.float64)
    lo = (c64 - hi - mid).astype(_BF16_NP)
    return hi.astype(_BF16_NP), mid.astype(_BF16_NP), lo


def _prepare(vertices, faces):
    """Host binning/pruning/packing. Returns (caps, in_maps data, slot maps)."""
    planes, valid, xsmin, xsmax, ysmin, ysmax, zmin_tri = _planes64(vertices, faces)
    ntj = _W // _TJ

    kept_lists = []  # [core][slot_ordering later] per-tile candidate arrays
    for c in range(8):
        b, h = c // 2, c % 2
        P = planes[b]  # [4, 3, F]
        tiles = []
        for t in range(_NT):
            tj, ti = t % ntj, t // ntj
            j0, i0 = tj * _TJ, ti * _TI + 64 * h
            xlo, xhi = 5 * j0 + 2.5, 5 * (j0 + _TJ - 1) + 2.5
            ylo, yhi = 5 * i0 + 2.5, 5 * (i0 + _TI - 1) + 2.5
            cand = np.where(valid[b] & (xsmax[b] >= xlo) & (xsmin[b] <= xhi)
                            & (ysmax[b] >= ylo) & (ysmin[b] <= yhi))[0]
            if len(cand) == 0:
                tiles.append((t, np.empty(0, np.int64)))
                continue
            corners = np.array(
                [[j0, i0, 1], [j0 + _TJ - 1, i0, 1],
                 [j0, i0 + _TI - 1, 1], [j0 + _TJ - 1, i0 + _TI - 1, 1]],
                np.float64)
            Wc = corners @ P[3][:, cand]           # [4, nc]
            zlo = np.maximum(Wc.min(0), zmin_tri[b][cand])
            covers = np.ones(len(cand), bool)
            for k in range(3):
                Pc = corners @ P[k][:, cand]
                covers &= (Pc <= _OFF - _S * _COVER_MARGIN).all(axis=0)
            bound = (Wc.max(0)[covers].min() + _BOUND_MARGIN) if covers.any() else np.inf
            keep = zlo <= bound
            order = cand[keep][np.argsort(zlo[keep])]
            tiles.append((t, order))
        kept_lists.append(tiles)

    # sort each core's tiles by kept desc -> slots; per-rank max across cores
    slot_orders = []
    for c in range(8):
        order = sorted(range(_NT), key=lambda t: -len(kept_lists[c][t][1]))
        slot_orders.append(order)
    caps = []
    for s in range(_NT):
        m = max(len(kept_lists[c][slot_orders[c][s]][1]) for c in range(8))
        caps.append(max(32, ((m + 31) // 32) * 32))
    caps = tuple(int(min(c, 512)) for c in caps)
    assert all(caps[i] >= caps[i + 1] for i in range(_NT - 1))
    total = sum(caps)

    in_maps = []
    for c in range(8):
        b, h = c // 2, c % 2
        coef_g64 = np.zeros((12, total), np.float64)
        coef_g64[np.arange(4) * 3 + 2, :] = _BIGC  # dummy padding planes
        pix_g = np.zeros((36, _NT * 128), np.float32)
        off = 0
        for s in range(_NT):
            t = slot_orders[c][s]
            idx = kept_lists[c][t][1]
            n = len(idx)
            for k in range(4):
                coef_g64[3 * k : 3 * k + 3, off : off + n] = planes[b, k][:, idx]
            off += caps[s]
            tj, ti = t % (_W // _TJ), t // (_W // _TJ)
            j0, i0 = tj * _TJ, ti * _TI + 64 * h
            jj = j0 + np.tile(np.arange(_TJ, dtype=np.float32), _TI)
            ii = i0 + np.repeat(np.arange(_TI, dtype=np.float32), _TJ)
            for r in range(12):
                pix_g[3 * r + 0, s * 128 : (s + 1) * 128] = jj
                pix_g[3 * r + 1, s * 128 : (s + 1) * 128] = ii
                pix_g[3 * r + 2, s * 128 : (s + 1) * 128] = 1.0
        coef36 = np.zeros((36, total), _BF16_NP)
        for k in range(4):
            hi, mid, lo = _split3(coef_g64[3 * k : 3 * k + 3])
            coef36[9 * k + 0 : 9 * k + 3] = hi
            coef36[9 * k + 3 : 9 * k + 6] = mid
            coef36[9 * k + 6 : 9 * k + 9] = lo
        pix36 = pix_g.astype(_BF16_NP)
        # dense [128, total + NT*128] layout: limb rows at partitions 32k..32k+8
        data = np.zeros((128, total + _NT * 128), _BF16_NP)
        for k in range(4):
            data[32 * k : 32 * k + 9, :total] = coef36[9 * k : 9 * k + 9]
            data[32 * k : 32 * k + 9, total:] = pix36[9 * k : 9 * k + 9]
        in_maps.append({"data": data})
    return caps, in_maps, slot_orders


def kernel(vertices, faces):
    vertices = np.asarray(vertices)
    faces = np.asarray(faces)
    caps, in_maps, slot_orders = _prepare(vertices, faces)

    nc = _get_nc(caps)
    kw = dict(PROFILE.get("run_kwargs", {}))
    res = run_bass_kernel_spmd(nc, in_maps, list(range(8)), **kw)
    PROFILE["last_result"] = res

    ntj = _W // _TJ
    out = np.empty((_B, _H, _W), np.float32)
    for c in range(8):
        b, h = c // 2, c % 2
        z = res.results[c]["out"]  # [128, NT]
        for s in range(_NT):
            t = slot_orders[c][s]
            tj, ti = t % ntj, t // ntj
            j0, i0 = tj * _TJ, ti * _TI + 64 * h
            out[b, i0 : i0 + _TI, j0 : j0 + _TJ] = z[:, s].reshape(_TI, _TJ)
    return out
